# revision 84
# baseline (speedup 1.0000x reference)
"""Multi-head attention kernel for Trainium2, sharded over 8 NeuronCores.

Sharding: data parallel over batch (B=2 -> 4 cores each) x tensor parallel
over heads (12 heads -> 3 heads per core). Each core computes QKV projections,
attention, and a partial output projection for its 3 heads; the per-head
partial output projections are summed on the host (the all-reduce of the
tensor-parallel hint, done during the gather step) and the output bias added.

Design (per core):
  - All matmul operands are fp16 (x, W*, q, k, ctx, Wo) -- halves input DMA
    vs fp32 at full matmul rate; accumulations stay fp32 in PSUM. Partial
    outputs return as fp16 (summed in fp32 on the host).
  - x arrives pre-transposed and pre-tiled as xT [128, 6, 2048] so the
    contraction dim (d) sits on SBUF partitions for all QKV matmuls.
  - q and k are produced head-by-head directly in transposed form via a
    stacked weight [Wq_h | Wk_h]; scores are computed transposed
    (scoresT [s_k, s_q]) so the softmax probabilities feed the probs@V
    matmul with no transpose.
  - probs are written by the scalar engine as fp8e4m3 = exp(s/8 - 2); the
    -2 shift keeps the distribution inside e4m3's range and cancels exactly
    in the softmax ratio.
  - probs@V runs in fp8 DoubleRow perf mode (two 128-deep contraction
    chunks per matmul at 2x rate). v rides as an fp8 hi+lo pair
    (lo = fp8(v - fp8(v))) accumulated into the same PSUM group, so v keeps
    ~fp16 accuracy while both DoubleRow operands are fp8.
  - softmax denominators ride for free: the v-hi operand carries a block of
    ones columns (zeros in v-lo), so rows 64..127 of the probs@V
    accumulation are the per-query sums of the same quantized probs --
    quantization partially cancels in the ratio.
  - the v bias never touches the device: ctx_h = sum(p v)/sum(p) + bv_h, so
    sum_h bv_h @ Wo_h folds into the output bias on the host.
  - with probs@V halved by DoubleRow the kernel is ACT(exp)-bound, so the
    emission scheduler paces one [128,1024] exp per ~1038ns slot and packs
    all other PE work (qk/v/out projections, lagged DoubleRow matmuls) into
    the leftover budget of each slot, deadline-ordered. Blocks run
    head-major so each phase only needs its own head's q/k projections.
"""

import heapq
from itertools import count

import numpy as np

import concourse.mybir as mybir
from concourse import bacc
from concourse.tile import TileContext
from concourse.bass_utils import run_bass_kernel_spmd

H, D, DH = 12, 768, 64
B, S = 2, 2048
NCORES = 8
CORES_PER_BATCH = 4
HPC = 3  # heads per core
SQ = 512  # query-chunk width
NSQ = S // SQ  # 4
NSK = S // 128  # 16 key chunks
NPR = NSK // 2  # 8 key-chunk pairs
NDC = D // 128  # 6 contraction chunks
VW = HPC * DH  # 192 v columns per core
NBLK = HPC * NSQ  # 12 attention blocks, head-major: b = h*NSQ + sc

F32 = mybir.dt.float32
F16 = mybir.dt.float16
F8 = mybir.dt.float8e4
ADD = mybir.AluOpType.add
SUB = mybir.AluOpType.subtract
MULT = mybir.AluOpType.mult
EXP = mybir.ActivationFunctionType.Exp
DRM = mybir.MatmulPerfMode.DoubleRow
EXP_SHIFT = -2.0  # probs = exp(s/8 - 2); cancels in the softmax ratio

# cost-model pacing constants (ns)
SLOT = 1038.0  # one [128,1024] exp on ACT
C_SCORE = 426.0  # two [128,512] fp16 score matmuls
C_QKMM = 213.0
C_VUNIT = 480.0
C_DR = 214.0  # one hi+lo DoubleRow pair
C_PROJ = 640.0


def _build_module():
    nc = bacc.Bacc("TRN2", target_bir_lowering=False, debug=False, num_devices=NCORES)
    xT = nc.declare_dram_parameter("xT", [128, NDC, S], F16, isOutput=False)
    wqk = nc.declare_dram_parameter("wqk", [128, HPC, NDC, 128], F16, isOutput=False)
    wv = nc.declare_dram_parameter("wv", [128, NDC, VW], F16, isOutput=False)
    wo01 = nc.declare_dram_parameter("wo01", [128, D], F16, isOutput=False)
    wo2 = nc.declare_dram_parameter("wo2", [64, D], F16, isOutput=False)
    bqk = nc.declare_dram_parameter("bqk", [128, HPC], F32, isOutput=False)
    out = nc.declare_dram_parameter("out", [S, D], F16, isOutput=True)

    with TileContext(nc) as tc:
        _body(nc, tc, xT, wqk, wv, wo01, wo2, bqk, out)
    nc.compile()
    return nc


def _body(nc, tc, xT, wqk, wv, wo01, wo2, bqk, out):
    with (
        tc.tile_pool(name="persist", bufs=1) as P1,
        tc.tile_pool(name="work", bufs=4) as W2,
        tc.tile_pool(name="probs", bufs=3) as PR,
        # PSUM budget is 8 banks of [128, 512] fp32:
        #   ACC: one shared rotating pool for qk-proj, v-proj, ctx accum and
        #        out-proj tiles (4 banks)
        #   SPS: [128, 1024] score tiles, double-buffered (4 banks) -- pairs
        #        of key-chunks share one tile so exp runs 1024 wide
        tc.tile_pool(name="acc", bufs=4, space="PSUM") as ACC,
        tc.tile_pool(name="sps", bufs=2, space="PSUM") as SPS,
    ):
        xT_sb = P1.tile([128, NDC, S], F16, tag="xT")
        wqk_sb = P1.tile([128, HPC, NDC, 128], F16, tag="wqk")
        wv_sb = P1.tile([128, NDC, VW], F16, tag="wv")
        wo01_sb = P1.tile([128, D], F16, tag="wo01")
        wo2_sb = P1.tile([64, D], F16, tag="wo2")
        bqk_sb = P1.tile([128, HPC], F32, tag="bqk")
        ebias = P1.tile([128, 1], F32, tag="ebias")
        qT = [P1.tile([64, S], F16, tag=f"qT{h}", name=f"qT{h}") for h in range(HPC)]
        kT = [P1.tile([64, S], F16, tag=f"kT{h}", name=f"kT{h}") for h in range(HPC)]
        # v hi/lo fp8 tiles: per chunk-pair c and parity j, per head a
        # [v (64) | ones (64)] block (ones in hi, zeros in lo)
        vph = P1.tile([128, NPR, 2, HPC * 128], F8, tag="vph")
        vpl = P1.tile([128, NPR, 2, HPC * 128], F8, tag="vpl")

        # DMAs in first-needed order. Each dma_start pays ~1.3us of
        # serialized HWDGE/DGE overhead plus 0.9us sem latency, so batch
        # big -- except the first qk unit's dependencies (wqk head 0, the
        # first query-chunk of xT split into column halves) so the score ->
        # exp stream starts as early as the input bandwidth allows.
        nc.sync.dma_start(wqk_sb[:, 0, :, :], wqk[:, 0, :, :])
        nc.sync.dma_start(xT_sb[:, :, 0:256], xT[:, :, 0:256])
        nc.sync.dma_start(xT_sb[:, :, 256:512], xT[:, :, 256:512])
        nc.sync.dma_start(bqk_sb[:], bqk[:])
        for sc in range(1, NSQ):
            nc.sync.dma_start(
                xT_sb[:, 0:3, sc * SQ:(sc + 1) * SQ],
                xT[:, 0:3, sc * SQ:(sc + 1) * SQ],
            )
            nc.sync.dma_start(
                xT_sb[:, 3:6, sc * SQ:(sc + 1) * SQ],
                xT[:, 3:6, sc * SQ:(sc + 1) * SQ],
            )
        nc.sync.dma_start(wv_sb[:], wv[:])
        nc.sync.dma_start(wqk_sb[:, 1:3, :, :], wqk[:, 1:3, :, :])
        nc.sync.dma_start(wo01_sb[:], wo01[:])
        nc.sync.dma_start(wo2_sb[:], wo2[:])
        nc.vector.memset(ebias[:], EXP_SHIFT)
        # ones columns next to each head's v-hi block (softmax denominator
        # trick); the v-lo ones-region must be zero (no double count)
        nc.gpsimd.memset(
            vph[:].rearrange("p c j (h m) -> p c j h m", m=128)[:, :, :, :, 64:128],
            1.0,
        )
        nc.gpsimd.memset(
            vpl[:].rearrange("p c j (h m) -> p c j h m", m=128)[:, :, :, :, 64:128],
            0.0,
        )

        # ---- emission scheduler state -------------------------------------
        emitted = set()  # readiness flags
        heapQ = []  # qk items: may pre-empt a slot's score matmuls
        heapM = []  # everything else: only ever emitted after the scores
        pending = []  # items whose flags aren't satisfied yet
        seq = count()
        credit = [0.0]
        slot = [0]

        def add(deadline, cost, fn, needs=(), pre=False, gated=False):
            # gated: DMA-fed work that must not be emitted before its
            # deadline (the in-order PE queue would stall on the transfer)
            item = (deadline, next(seq), cost, tuple(needs), fn, pre, gated)
            if all(f in emitted for f in item[3]):
                heapq.heappush(heapQ if pre else heapM, item)
            else:
                pending.append(item)

        def refresh():
            still = []
            for item in pending:
                if all(f in emitted for f in item[3]):
                    heapq.heappush(heapQ if item[5] else heapM, item)
                else:
                    still.append(item)
            pending[:] = still

        def pump_pre():
            # before a slot's scores: only qk work (which the upcoming
            # scores depend on) may run; forced when overdue
            while heapQ:
                deadline, _, cost, _, fn, _, gated = heapQ[0]
                if deadline > slot[0] and (gated or credit[0] < cost):
                    break
                heapq.heappop(heapQ)
                fn()
                credit[0] -= cost
                refresh()

        def pump(spend_cap=None):
            # after the slot's scores+exp: drain both queues in deadline
            # order; overdue items ignore credit AND the cap, but credit-
            # funded (early) items stop once the slot's PE budget is spent
            spent = 0.0
            while True:
                pick = None
                for h in (heapQ, heapM):
                    if h and (pick is None or h[0][0] < pick[0][0]):
                        pick = h
                if pick is None:
                    break
                deadline, _, cost, _, fn, _, gated = pick[0]
                if deadline <= slot[0]:
                    pass
                elif (
                    (not gated or deadline <= slot[0] + 1)
                    and credit[0] >= cost
                    and (spend_cap is None or spent + cost <= spend_cap)
                ):
                    pass
                else:
                    break
                heapq.heappop(pick)
                fn()
                credit[0] -= cost
                spent += cost
                refresh()

        # ---- work units ---------------------------------------------------
        qk_ps = {}

        def unit00_tt(ch):
            ps = qk_ps[0, 0, ch]
            c0 = ch * 256
            nc.vector.tensor_tensor(
                kT[0][:, c0:c0 + 256],
                ps[64:128, :],
                bqk_sb[64:128, 0:1].to_broadcast([64, 256]),
                ADD,
            )
            nc.vector.tensor_tensor(
                qT[0][:, c0:c0 + 256],
                ps[0:64, :],
                bqk_sb[0:64, 0:1].to_broadcast([64, 256]),
                ADD,
            )
            emitted.add(f"qk0_0{'ab'[ch]}")
            if ch == 1:
                emitted.add("qk0_0")

        def qk_mm(h, q, o, ch=None):
            # the first unit's column halves use separate PSUM tiles: a
            # shared tile would serialize the b-half group behind the
            # a-half's bias-add readers
            if o == 0 and ch is None:
                qk_ps[h, q] = ACC.tile([128, SQ], F32, tag="acc", name=f"qkps{h}_{q}")
            elif o == 0:
                qk_ps[h, q, ch] = ACC.tile(
                    [128, 256], F32, tag="acc", name=f"qkps{h}_{q}_{ch}"
                )
            ps = qk_ps[h, q] if ch is None else qk_ps[h, q, ch]
            c0, cw = (0, SQ) if ch is None else (ch * 256, 256)
            nc.tensor.matmul(
                ps[:] if ch is not None else ps[:, c0:c0 + cw],
                wqk_sb[:, h, o, :],
                xT_sb[:, o, q * SQ + c0:q * SQ + c0 + cw],
                start=(o == 0),
                stop=(o == NDC - 1),
            )
            if ch is not None:
                # the first unit runs as two column-half accumulation groups
                # so the first 256 keys/queries release as soon as the first
                # half-slab DMA lands
                # the b-half's bias-adds are a separate scheduler item:
                # they must not precede exp-a in emission order
                # (cross-engine deps are monotonic per-engine counters)
                if o == NDC - 1:
                    if ch == 0:
                        unit00_tt(0)
                    else:
                        emitted.add("qk0_0b_mms")
                return
            if o == NDC - 1:
                # bias adds; for head-0 units the kT side lands first (the
                # score stream consumes kT chunks in order, and these units
                # are DMA-gated), split in halves so the first chunks
                # release as early as possible
                def k_half(c0, cw):
                    nc.vector.tensor_tensor(
                        kT[h][:, q * SQ + c0:q * SQ + c0 + cw],
                        ps[64:128, c0:c0 + cw],
                        bqk_sb[64:128, h:h + 1].to_broadcast([64, cw]),
                        ADD,
                    )

                def q_full():
                    nc.vector.tensor_tensor(
                        qT[h][:, q * SQ:(q + 1) * SQ],
                        ps[0:64, :],
                        bqk_sb[0:64, h:h + 1].to_broadcast([64, SQ]),
                        ADD,
                    )

                if h == 0 and q == 0:
                    # the very first scores need kT chunks 0-1 AND qT
                    k_half(0, 256)
                    q_full()
                    k_half(256, 256)
                elif h == 0:
                    # kT chunks are consumed (by block (0,0)) before this
                    # unit's own qT slice (block (0,q))
                    k_half(0, 256)
                    k_half(256, 256)
                    q_full()
                else:
                    k_half(0, SQ)
                    q_full()
                emitted.add(f"qk{h}_{q}")

        def v_unit(mk):
            # one key-chunk of v = xT.T @ [Wv_h0|Wv_h1|Wv_h2], split into
            # fp8 hi + lo (no bias: bv folds into bo on the host)
            c, j = mk // 2, mk % 2
            ps = ACC.tile([128, VW], F32, tag="acc", name=f"vps{mk}")
            for o in range(NDC):
                nc.tensor.matmul(
                    ps[:],
                    xT_sb[:, o, mk * 128:(mk + 1) * 128],
                    wv_sb[:, o, :],
                    start=(o == 0),
                    stop=(o == NDC - 1),
                )
            hi = vph[:, c, j, :].rearrange("p (h m) -> p h m", m=128)[:, :, 0:64]
            lo = vpl[:, c, j, :].rearrange("p (h m) -> p h m", m=128)[:, :, 0:64]
            psv = ps[:].rearrange("p (h m) -> p h m", m=64)
            nc.vector.tensor_copy(hi, psv)
            nc.vector.tensor_tensor(lo, psv, hi, SUB)
            emitted.add(f"v{mk}")

        probs_t = {}
        cps_t = {}
        ctxs = [
            (
                W2.tile([128, SQ], F16, tag="ctx01", name=f"c01_{sc}"),
                W2.tile([64, SQ], F16, tag="ctx2", name=f"c2_{sc}"),
            )
            for sc in range(NSQ)
        ]

        def dr_pair(b, c):
            h, sc = b // NSQ, b % NSQ
            if c == 0:
                cps_t[b] = ACC.tile([128, SQ], F32, tag="acc", name=f"cps{b}")
            cps = cps_t[b]
            pr = probs_t[b][:, c * 2 * SQ:(c + 1) * 2 * SQ].rearrange(
                "p (j n) -> p j n", j=2
            )
            nc.tensor.matmul(
                cps[:], vph[:, c, :, h * 128:(h + 1) * 128], pr,
                start=(c == 0), stop=False, perf_mode=DRM,
            )
            nc.tensor.matmul(
                cps[:], vpl[:, c, :, h * 128:(h + 1) * 128], pr,
                start=False, stop=(c == NPR - 1), perf_mode=DRM,
            )
            emitted.add(f"dr{b}_{c}")
            if c == NPR - 1:
                finish_block(b)

        def finish_block(b):
            # rows 0..63: unnormalized ctxT; rows 64..127: denominators
            h, sc = b // NSQ, b % NSQ
            cps = cps_t.pop(b)
            ctx01, ctx2 = ctxs[sc]
            last = b == NBLK - 1
            # last block: one wide reciprocal (658ns beats 4x258) then
            # per-ms multiplies so each out-projection piece releases early
            pieces = 4 if last else 1
            w = SQ // pieces
            r = W2.tile([64, SQ], F32, tag="recip", name=f"r{b}")
            nc.vector.reciprocal(r[:], cps[64:128, :])
            dst = ctx01[h * 64:(h + 1) * 64, :] if h < 2 else ctx2[:]
            for i in range(pieces):
                nc.vector.tensor_tensor(
                    dst[:, i * w:(i + 1) * w], cps[0:64, i * w:(i + 1) * w],
                    r[:, i * w:(i + 1) * w], MULT,
                )
                for ms in range(i * w // 128, (i + 1) * w // 128):
                    emitted.add(f"ctx{b}_{ms}")
            emitted.add(f"blk{b}")

        ot_t = {}

        def proj_piece(sc, ms, n0, nw):
            # half of out[sc,ms] = ctx01.T @ Wo01 + ctx2.T @ Wo2, as fp16
            ctx01, ctx2 = ctxs[sc]
            tail = sc == NSQ - 1
            row = (sc * 4 + ms) * 128
            if n0 == 0:
                ot_t[sc, ms] = W2.tile([128, D], F16, tag="out", name=f"ot{sc}_{ms}")
            ot = ot_t[sc, ms]
            ops_t = ACC.tile([128, nw], F32, tag="acc", name=f"ops{sc}_{ms}_{n0}")
            nc.tensor.matmul(
                ops_t[:], ctx01[:, ms * 128:(ms + 1) * 128],
                wo01_sb[:, n0:n0 + nw], start=True, stop=False,
            )
            nc.tensor.matmul(
                ops_t[:], ctx2[:, ms * 128:(ms + 1) * 128],
                wo2_sb[:, n0:n0 + nw], start=False, stop=True,
            )
            if tail and (ms < 2 or (ms == 3 and nw == 256)):
                # tail: the exp stream is over -- split whole-ms copies
                # between the idle ACT (ms0,1) and DVE (ms2,3) so the
                # serial copy chains run in parallel
                nc.scalar.activation(
                    ot[:, n0:n0 + nw], ops_t[:],
                    mybir.ActivationFunctionType.Copy,
                )
            else:
                nc.vector.tensor_copy(ot[:, n0:n0 + nw], ops_t[:])
            if n0 != 0:
                nc.sync.dma_start(out[row:row + 128, :], ot[:])

        # ---- static work list ---------------------------------------------
        # qk unit (h, q): six matmuls; needed (kT side) by pair 2q of block
        # (h, 0) at slot h*32 + 2q, minus slack for the DVE bias-add.
        for ch in range(2):
            for o in range(NDC):
                add(ch - 1, C_QKMM / 2,
                    lambda o=o, ch=ch: qk_mm(0, 0, o, ch=ch), pre=True,
                    gated=True)
        add(1, 0.0, lambda: unit00_tt(1), needs=("qk0_0b_mms",), pre=True)
        for h in range(HPC):
            for q in range(NSQ):
                if h == 0 and q == 0:
                    continue  # emitted above as column-half items
                if h == 0:
                    # head-0 units are gated by the xT slab DMAs; fractional
                    # deadlines emit them right after the slot's scores so
                    # they never stall the in-order PE queue on a transfer
                    d = 2 * q - (3 if q == 2 else 1)
                elif h == 2:
                    # head-2 units fit in the h1 phase's slack (the h2 phase
                    # carries the out-projections)
                    d = 48 + 2 * q
                else:
                    d = h * 32 + 2 * q - 3 - (1 if q == 0 else 0)
                for o in range(NDC):
                    # stagger the six matmuls so overdue-forcing never
                    # bursts a whole 1.3us unit into one slot (head-0 stays
                    # tighter: its xT slabs arrive just-in-time)
                    stag = (0.5 if o < 3 else 0.25) if h == 0 else (NDC - 1 - o)
                    add(d - stag, C_QKMM, lambda h=h, q=q, o=o: qk_mm(h, q, o),
                        pre=True, gated=h == 0)
        # v units: needed by the lagged DoubleRow of block 0 onward; gated
        # so credit never pops them before the wv DMA has landed
        for mk in range(NSK):
            add(7 + mk, C_VUNIT, lambda mk=mk: v_unit(mk), gated=True)
        # DoubleRow probs@V pairs: blocks 0..3 (head-0 phase) may lag up to
        # the probs-buffer deadline (3 buffers); later blocks pair-lag so
        # ctx/proj complete in-phase.
        for b in range(NBLK):
            for c in range(NPR):
                if b < NSQ:
                    d = 8 * (b + 3) - 1
                elif b == NBLK - 1:
                    # the last block has no successor to overload: drain its
                    # DoubleRow pairs in-block so the tail starts sooner
                    d = 8 * b + c + 2
                else:
                    d = 8 * b + c + 10
                add(d, C_DR, lambda b=b, c=c: dr_pair(b, c),
                    needs=(f"exp{b}_{c}", f"v{2 * c}", f"v{2 * c + 1}"))
        # out projections: after the h2 block of sc completes ctx2 (and
        # ctx01 long before). sc=3 lands in the tail, per-ms pipelined.
        for sc in range(NSQ):
            b2 = 2 * NSQ + sc
            for ms in range(4):
                # the tail (sc3) emits in expected-completion order so no
                # output DMA's SEQ-held wait blocks the next DMA's issue
                mo = [0, 2, 1, 3].index(ms) if sc == NSQ - 1 else ms
                for n0, nw in ((0, 512), (512, 256)):
                    add(8 * (b2 + 1) + 2 + 2 * mo + (1 if n0 else 0), C_PROJ / 2,
                        lambda sc=sc, ms=ms, n0=n0, nw=nw: proj_piece(sc, ms, n0, nw),
                        needs=(f"blk{NSQ + sc}", f"ctx{b2}_{ms}"))

        # PE warmup: the cost model's p-state ramp needs ~3us of sustained
        # matmul activity for full clock; the first real matmuls wait on DMA
        # anyway, so burn the wait on narrow dummy matmuls (128-wide: cheap
        # to preempt) that bridge the gap until the first xT slab lands.
        warm = P1.tile([64, 512], F16, tag="warm")
        nc.vector.memset(warm[:].bitcast(F32), 0.0)
        wps = ACC.tile([128, 128], F32, tag="acc", name="warmps")
        for _ in range(32):
            nc.tensor.matmul(
                wps[:], warm[:, 0:128], warm[:, 0:128], start=True, stop=True
            )
        # pre-load the ACT exp table set during the same dead time
        wact = P1.tile([64, 1], F16, tag="wact")
        nc.scalar.activation(wact[:], warm[:, 0:2].bitcast(F32), EXP, scale=0.125)

        # ---- slot loop: one exp per slot, budget-paced fillers. Scores are
        # emitted one slot ahead of their exp so each slot's PE stream leads
        # with the matmuls ACT is about to need (filler overruns then only
        # delay fillers, not the exp cadence).
        pairs = [(b, j) for b in range(NBLK) for j in range(NPR)]
        sps_t = {}

        def emit_exp(p):
            b, j = pairs[p]
            nc.scalar.activation(
                probs_t[b][:, j * 2 * SQ:(j + 1) * 2 * SQ], sps_t.pop(p)[:],
                EXP, scale=0.125, bias=ebias[:],
            )
            emitted.add(f"exp{b}_{j}")
            refresh()

        def half_pair0(qh):
            # q-halved first pair: scores+exp on a 256-query half release
            # the activation stream one DMA-half earlier
            sps = sps_t[0]
            for half in range(2):
                nc.tensor.matmul(
                    sps[:, half * SQ + qh * 256:half * SQ + qh * 256 + 256],
                    kT[0][:, half * 128:(half + 1) * 128],
                    qT[0][:, qh * 256:qh * 256 + 256],
                    start=True,
                    stop=True,
                )
            nc.scalar.activation(
                probs_t[0][:, 0:2 * SQ]
                .rearrange("p (j n) -> p j n", j=2)[:, :, qh * 256:qh * 256 + 256],
                sps[:].rearrange("p (j n) -> p j n", j=2)
                [:, :, qh * 256:qh * 256 + 256],
                EXP, scale=0.125, bias=ebias[:],
            )
            if qh == 1:
                emitted.add("exp0_0")
                refresh()

        for p, (b, j) in enumerate(pairs):
            h, sc = b // NSQ, b % NSQ
            if j == 0:
                probs_t[b] = PR.tile([128, NSK * SQ], F8, tag="probs", name=f"pr{b}")
            pump_pre()  # qk deps for the upcoming scores
            if p == 0:
                sps_t[0] = SPS.tile([128, 2 * SQ], F32, tag="sps", name="sps0")
                half_pair0(0)
                credit[0] = min(credit[0] + SLOT - C_SCORE, 4 * SLOT)
                pump(spend_cap=SLOT - C_SCORE)
                slot[0] += 1
                continue
            if p == 1:
                half_pair0(1)
            sps = sps_t[p] = SPS.tile(
                [128, 2 * SQ], F32, tag="sps", name=f"sps{b}_{j}"
            )
            for half in range(2):
                mk = 2 * j + half
                nc.tensor.matmul(
                    sps[:, half * SQ:(half + 1) * SQ],
                    kT[h][:, mk * 128:(mk + 1) * 128],
                    qT[h][:, sc * SQ:(sc + 1) * SQ],
                    start=True,
                    stop=True,
                )
            if p > 1:
                emit_exp(p - 1)
            credit[0] = min(credit[0] + SLOT - C_SCORE, 4 * SLOT)
            pump(spend_cap=SLOT - C_SCORE)
            slot[0] += 1
        emit_exp(len(pairs) - 1)
        # tail: drain everything left (last block's DR pairs, ctx, proj sc=3)
        credit[0] = 1e9
        slot[0] = 10 ** 6
        while heapQ or heapM or pending:
            n0 = len(heapQ) + len(heapM) + len(pending)
            pump()
            if len(heapQ) + len(heapM) + len(pending) == n0:
                raise RuntimeError(
                    f"scheduler deadlock: {len(heapQ)}+{len(heapM)} heap / "
                    f"{len(pending)} pending"
                )


_CACHE = {}


def _get_module():
    if "nc" not in _CACHE:
        _CACHE["nc"] = _build_module()
    return _CACHE["nc"]


def make_in_maps(x, Wq, Wk, Wv, bq, bk, bv, Wo):
    f16 = np.float16
    in_maps = []
    for c in range(NCORES):
        b = c // CORES_PER_BATCH
        hh = [HPC * (c % CORES_PER_BATCH) + i for i in range(HPC)]
        # xT pre-tiled to [128, 6, 2048]: partition p, d-chunk o, seq s
        xt = x[b].T.reshape(NDC, 128, S).transpose(1, 0, 2)
        # wqk pre-tiled to [128, 3, 6, 128]
        wqk = np.stack(
            [np.concatenate([Wq[h], Wk[h]], axis=1) for h in hh]
        )  # [3, 768, 128]
        wqk = wqk.reshape(HPC, NDC, 128, 128).transpose(2, 0, 1, 3)
        # wv pre-tiled to [128, 6, 192]
        wv_stack = np.concatenate([Wv[h] for h in hh], axis=1)  # [768, 192]
        wv_stack = wv_stack.reshape(NDC, 128, VW).transpose(1, 0, 2)
        in_maps.append({
            "xT": np.ascontiguousarray(xt).astype(f16),
            "wqk": np.ascontiguousarray(wqk).astype(f16),
            "wv": np.ascontiguousarray(wv_stack).astype(f16),
            "wo01": np.ascontiguousarray(
                Wo[hh[0] * DH:(hh[0] + 2) * DH, :]
            ).astype(f16),
            "wo2": np.ascontiguousarray(
                Wo[hh[2] * DH:(hh[2] + 1) * DH, :]
            ).astype(f16),
            "bqk": np.ascontiguousarray(
                np.stack([np.concatenate([bq[h], bk[h]]) for h in hh], axis=1)
            ).astype(np.float32),
        })
    return in_maps


def gather(results, bv, Wo, bo):
    # ctx_h = softmax(scores) @ v_nobias + bv_h, so the bv contribution to
    # the output is a constant row: sum_h bv_h @ Wo_h, folded into bo here.
    bo_eff = bo.astype(np.float64) + bv.reshape(-1).astype(np.float64) @ Wo.astype(
        np.float64
    )
    out = np.empty((B, S, D), np.float32)
    for b in range(B):
        acc = results[b * CORES_PER_BATCH]["out"].astype(np.float64, copy=True)
        for c in range(b * CORES_PER_BATCH + 1, (b + 1) * CORES_PER_BATCH):
            acc += results[c]["out"].astype(np.float64)
        out[b] = (acc + bo_eff[None, :]).astype(np.float32)
    return out


def kernel(x, Wq, Wk, Wv, bq, bk, bv, Wo, bo, c=0, **_unused):
    x, Wq, Wk, Wv, bq, bk, bv, Wo, bo = (
        np.asarray(a, np.float32) for a in (x, Wq, Wk, Wv, bq, bk, bv, Wo, bo)
    )
    nc = _get_module()
    in_maps = make_in_maps(x, Wq, Wk, Wv, bq, bk, bv, Wo)
    res = run_bass_kernel_spmd(nc, in_maps, list(range(NCORES)))
    return gather(res.results, bv, Wo, bo)
